# revision 8
# baseline (speedup 1.0000x reference)
"""Multi-head attention (B=2, S=2048, D=1024, H=16) on 8 Trainium2 cores.

Sharding: 2-way data parallel on batch x 4-way tensor parallel on heads.
Core c handles batch b = c // 4 and head group g = c % 4 (4 heads, 256 dims).

Per-core pipeline (all "feature-major" / transposed layouts so that every
matmul streams a long free dim and biases land on partitions):
  XT [1024, 2048]  (host-transposed input slice, bf16)
  QT = (Wq.T @ X.T + bq) / 8   [256, 2048]
  KT =  Wk.T @ X.T + bk        [256, 2048]
  V  =  X @ Wv + bv            [2048, 256]   (bias via ones-row matmul)
  per head h:
    scoresT[t, s] = KT_h[:, t]_tile.T @ QT_h      (PE, K=64)
    expT = exp(scoresT)                           (ACT, PSUM -> SBUF bf16)
    hoT'[65, s]  = [V_h | 1].T @ expT             (PE, accumulate over t)
      rows 0:64 = unnormalized head out (d, s), row 64 = sumexp[s]
    normalization (engineered for fast PSUM release -- GPSIMD and DMA
    cannot read PSUM, ACT is backlogged with exp at block boundaries, so
    the two PSUM reads both go to DVE, then the rest trails off the
    critical path):
      sm  = pv[64]            (DVE copy -> partition 0)
      hoU = pv[0:64]          (DVE copy, unnormalized, bf16) -> pv FREE
      sr  = 1/sm              (DVE approx reciprocal)
      rbc = bcast(sr, 64)     (GPSIMD partition broadcast)
      hoT = hoU * rbc         (DVE, gates only the out-projection)
  outT[e, s] = Wout_g partials: lhsT=Wout_g tiles, rhs=hoT
Host sums the 4 per-group partials per batch and transposes back.

Scheduling: ~22 warmup matmuls bridge the initial DMA window so the PE
HAM clock-gate un-throttles before real work; scores of head h+1 ride
inside head h's attnV per-t (exp tiles released after 4 back-to-back
matmuls); dummy matmuls bridge each block's ~4.5us PSUM-release window
so the HAM gate never sees a full idle window; the out-projection is ch
outer and gated on fine (512) norm chunks of head 3 via s-half-split
hoT tiles.
"""

import numpy as np
import ml_dtypes

BF16 = ml_dtypes.bfloat16

S = 2048  # sequence length
C = 1024  # d_model
NH = 16  # total heads
DK = 64  # head dim
N_CORES = 8
HPC = 4  # heads per core
DH = HPC * DK  # 256 per-core head dims
P = 128
VW = 72  # per-head stride in the V' buffer: 64 V cols + 1 ones col + 7 pad

_CACHE = {}


def _build_program():
    import concourse.bacc as bacc
    import concourse.mybir as mybir
    import concourse.tile as tile
    from contextlib import ExitStack

    dt = mybir.dt
    AF = mybir.ActivationFunctionType

    nc = bacc.Bacc("TRN2", target_bir_lowering=False, debug=False,
                   num_devices=N_CORES)

    xt = nc.dram_tensor("xt", [C, S], dt.bfloat16, kind="ExternalInput")
    wq = nc.dram_tensor("wq", [C, DH], dt.bfloat16, kind="ExternalInput")
    wk = nc.dram_tensor("wk", [C, DH], dt.bfloat16, kind="ExternalInput")
    wv = nc.dram_tensor("wv", [C, DH], dt.bfloat16, kind="ExternalInput")
    wo = nc.dram_tensor("wo", [DH, C], dt.bfloat16, kind="ExternalInput")
    # cols: [bq_tile0/8, bq_tile1/8, bk_tile0, bk_tile1]
    bqk = nc.dram_tensor("bqk", [P, 4], dt.float32, kind="ExternalInput")
    bv = nc.dram_tensor("bv", [1, DH], dt.bfloat16, kind="ExternalInput")
    outT = nc.dram_tensor("outT", [C, S], dt.bfloat16, kind="ExternalOutput")

    xt_r = xt.rearrange("(n p) s -> n p s", p=P)  # [8, 128, 2048]
    wq_r = wq.rearrange("(n p) d -> n p d", p=P)  # [8, 128, 256]
    wk_r = wk.rearrange("(n p) d -> n p d", p=P)
    wv_r = wv.rearrange("(n p) d -> n p d", p=P)
    wo_r = wo.rearrange("(n p) e -> n p e", p=P)  # [2, 128, 1024]
    outT_r = outT.rearrange("(n p) s -> n p s", p=P)  # [8, 128, 2048]

    with ExitStack() as ctx:
        tc = ctx.enter_context(tile.TileContext(nc))
        sb = ctx.enter_context(tc.tile_pool(name="sb", bufs=1))
        xpool = ctx.enter_context(tc.tile_pool(name="xpool", bufs=1))
        pool2 = ctx.enter_context(tc.tile_pool(name="pool2", bufs=1))
        spool = ctx.enter_context(tc.tile_pool(name="spool", bufs=2, space="PSUM"))
        vpool = ctx.enter_context(tc.tile_pool(name="vpool", bufs=1, space="PSUM"))

        # ---- persistent SBUF ----
        qt_sb = [sb.tile([P, S], dt.bfloat16, name=f"qt{i}", tag=f"qt{i}") for i in range(2)]
        kt_sb = [sb.tile([P, S], dt.bfloat16, name=f"kt{i}", tag=f"kt{i}") for i in range(2)]
        v_sb = [sb.tile([P, HPC * VW], dt.bfloat16, name=f"v{i}", tag=f"v{i}") for i in range(16)]
        exp_sb = [sb.tile([P, S], dt.bfloat16, name=f"e{i}", tag=f"e{i}") for i in range(16)]
        # hoT tiles split by (d-half, s-half) so the out-projection's first
        # chunk is gated only on the norm chunks covering its s range
        hot_sb = [[sb.tile([P, S // 2], dt.bfloat16, name=f"ho{i}{j}",
                           tag=f"ho{i}{j}") for j in range(2)] for i in range(2)]
        # unnormalized head-out, per (head, s-half) at base partition 0 --
        # the norm mul needs both SBUF inputs at equal base partitions
        hou_sb = [[sb.tile([DK, S // 2], dt.bfloat16, name=f"hu{i}{j}",
                           tag=f"hu{i}{j}") for j in range(2)] for i in range(HPC)]
        wo_sb = [sb.tile([P, C], dt.bfloat16, name=f"wo{i}", tag=f"wo{i}") for i in range(2)]
        bqk_sb = sb.tile([P, 4], dt.float32, name="bqk", tag="bqk")
        bv_sb = sb.tile([1, DH], dt.bfloat16, name="bv", tag="bv")
        ones_sb = sb.tile([1, P], dt.bfloat16, name="ones", tag="ones")
        warm_sb = sb.tile([P, 640], dt.bfloat16, name="warm", tag="warm")

        # ---- phase-1-only SBUF ----
        xt_sb = [xpool.tile([P, S], dt.bfloat16, name=f"x{i}", tag=f"x{i}") for i in range(8)]
        wq_sb = [xpool.tile([P, DH], dt.bfloat16, name=f"wq{i}", tag=f"wq{i}") for i in range(8)]
        wk_sb = [xpool.tile([P, DH], dt.bfloat16, name=f"wk{i}", tag=f"wk{i}") for i in range(8)]
        wv_sb = [xpool.tile([P, DH], dt.bfloat16, name=f"wv{i}", tag=f"wv{i}") for i in range(8)]

        # ---- warmup: the PE HAM clock-gate starts at half clock and needs
        # ~3.4us of sustained busy to un-throttle. The first real matmul
        # can't start until the xt DMA lands (~6us), so burn that window on
        # dummy matmuls over a memset tile -- the real stream then runs at
        # full clock from its first instruction. ----
        nc.vector.memset(warm_sb, 1.0)
        nc.vector.memset(ones_sb, 1.0)
        wt = vpool.tile([DK + 1, S], dt.float32, name="wt", tag="av")
        for i in range(26):
            nc.tensor.matmul(wt[:, 0:512], lhsT=warm_sb[:, 0:DK + 1],
                             rhs=warm_sb[:, 128:640], start=True, stop=True)

        # ---- loads: xt on the two HWDGE queues (sync + scalar), weights on
        # the gpsimd software DGE so no compute engine burns time on DMA
        # issue mid-phase. xt comes in s-column halves so the first QKT
        # chunk can start after half the transfer ----
        for half in range(2):
            cs = slice(half * 1024, (half + 1) * 1024)
            for i in range(8):
                eng = nc.sync if i % 2 == 0 else nc.scalar
                eng.dma_start(out=xt_sb[i][:, cs], in_=xt_r[i][:, cs])
        for i in range(8):
            nc.gpsimd.dma_start(out=wq_sb[i], in_=wq_r[i])
        nc.gpsimd.dma_start(out=bqk_sb, in_=bqk[:, :])
        nc.gpsimd.dma_start(out=bv_sb, in_=bv[:, :])
        for i in range(8):
            nc.gpsimd.dma_start(out=wk_sb[i], in_=wk_r[i])
        for i in range(8):
            nc.gpsimd.dma_start(out=wv_sb[i], in_=wv_r[i])
        for i in range(2):
            nc.sync.dma_start(out=wo_sb[i], in_=wo_r[i])
        # pre-set the per-head ones column in each V' tile (cols h*VW + DK)
        for t in range(16):
            col = v_sb[t].rearrange("p (h w) -> p h w", w=VW)[:, :, DK:DK + 1]
            nc.vector.memset(col, 1.0)

        def qkt_unit(d2, ch, qk):
            """One [128, 1024] chunk of QT or KT for d-tile d2. The
            1/sqrt(dk) scale is folded into Wq host-side, so the epilogue is
            a plain bias-add on DVE (keeps ACT free for exp)."""
            dst, w_sb, bias_col = (
                (qt_sb, wq_sb, 0) if qk == 0 else (kt_sb, wk_sb, 2)
            )
            ps = spool.tile([P, 1024], dt.float32, name="mm", tag="mm")
            for half in range(2):
                for c8 in range(8):
                    nc.tensor.matmul(
                        ps[:, half * 512:(half + 1) * 512],
                        lhsT=w_sb[c8][:, d2 * P:(d2 + 1) * P],
                        rhs=xt_sb[c8][:, ch * 1024 + half * 512:
                                      ch * 1024 + (half + 1) * 512],
                        start=(c8 == 0), stop=(c8 == 7),
                    )
            nc.vector.tensor_scalar_add(
                dst[d2][:, ch * 1024:(ch + 1) * 1024], ps,
                bqk_sb[:, bias_col + d2:bias_col + d2 + 1],
            )

        def vproj_t(t):
            # V tile t: [128, 256] + bias via ones-row; packed [64|1|pad] x4.
            # Ones columns were pre-set at startup; the epilogue is a single
            # strided DVE copy so PE stays the pacer.
            ps = spool.tile([P, DH], dt.float32, name="mm", tag="mm")
            for c8 in range(8):
                nc.tensor.matmul(
                    ps, lhsT=xt_sb[c8][:, t * P:(t + 1) * P],
                    rhs=wv_sb[c8], start=(c8 == 0), stop=False,
                )
            nc.tensor.matmul(ps, lhsT=ones_sb, rhs=bv_sb,
                             start=False, stop=True)
            dst = v_sb[t].rearrange("p (h w) -> p h w", w=VW)[:, :, 0:DK]
            src = ps.rearrange("p (h w) -> p h w", w=DK)
            nc.vector.tensor_copy(dst, src)

        def scores_t(h, t):
            half_idx = h // 2
            row0 = (h % 2) * DK
            kth = kt_sb[half_idx]
            qth = qt_sb[half_idx]
            for ch in range(2):
                ps = spool.tile([P, 1024], dt.float32, name="mm", tag="mm")
                for half in range(2):
                    s0 = ch * 1024 + half * 512
                    nc.tensor.matmul(
                        ps[:, half * 512:(half + 1) * 512],
                        lhsT=kth[row0:row0 + DK, t * P:(t + 1) * P],
                        rhs=qth[row0:row0 + DK, s0:s0 + 512],
                        start=True, stop=True,
                    )
                nc.scalar.activation(
                    exp_sb[t][:, ch * 1024:(ch + 1) * 1024], ps, AF.Exp
                )

        def dummies(dm, n):
            # PE filler over the warm tile: bridges known PE stalls so the
            # HAM gate never sees a low-duty window
            for i in range(n):
                nc.tensor.matmul(dm, lhsT=warm_sb[:, 0:DK + 1],
                                 rhs=warm_sb[:, 128:640],
                                 start=True, stop=True)

        def ldw_fill(n):
            # LDWEIGHTS-only PE filler: needs no PSUM target, so it can run
            # in the boundary window where every PSUM bank still has
            # pending readers. The next real matmul reloads its own
            # weights, so these clobber nothing.
            for i in range(n):
                nc.tensor.ldweights(weights=warm_sb[:, 0:P])

        def norm_chunks(h, pv, n_chunks):
            # see module docstring. Emission order matters: engine queues
            # are in-order, so the pv-releasing PSUM reads go first (split
            # across DVE and ACT -- ACT carries one hoU chunk despite its
            # exp backlog), recips next, and the muls (each gated on a
            # gpsimd broadcast) last so they never block the PSUM release.
            half_idx = h // 2
            row0 = (h % 2) * DK
            csz = S // n_chunks

            def chunk(ci):
                cs = slice(ci * csz, (ci + 1) * csz)
                sh = (ci * csz) // (S // 2)
                lo = (ci * csz) % (S // 2)
                return cs, sh, slice(lo, lo + csz)

            if h + 1 < HPC:
                srs = []
                for ci in range(n_chunks):
                    cs, sh, lcs = chunk(ci)
                    sm = pool2.tile([1, csz], dt.float32, name="sm",
                                    tag="sm", bufs=2)
                    nc.vector.tensor_copy(sm, pv[DK:DK + 1, cs])
                    if ci == n_chunks - 1:
                        nc.scalar.copy(hou_sb[h][sh][:, lcs], pv[0:DK, cs])
                    else:
                        nc.vector.tensor_copy(hou_sb[h][sh][:, lcs],
                                              pv[0:DK, cs])
                    srs.append(sm)
                for ci in range(n_chunks):
                    sr = pool2.tile([1, csz], dt.float32, name="sr",
                                    tag="sr", bufs=2)
                    nc.vector.reciprocal_approx_fast(sr, srs[ci])
                    srs[ci] = sr
                rbcs = []
                for ci in range(n_chunks):
                    rbc = pool2.tile([DK, csz], dt.float32, name="rbc",
                                     tag="rbc", bufs=2)
                    nc.gpsimd.partition_broadcast(rbc, srs[ci])
                    rbcs.append(rbc)
                for ci in range(n_chunks):
                    cs, sh, lcs = chunk(ci)
                    nc.vector.tensor_mul(
                        hot_sb[half_idx][sh][row0:row0 + DK, lcs],
                        hou_sb[h][sh][:, lcs], rbcs[ci])
            else:
                # tail: the out-projection gate is the priority -- no hoU
                # detour (mul reads PSUM directly), chunks interleaved in
                # pairs so hoT s-half 0 lands as early as possible
                sig = [("s", 0), ("r", 0), ("s", 1), ("r", 1), ("m", 0),
                       ("m", 1), ("s", 2), ("r", 2), ("s", 3), ("r", 3),
                       ("m", 2), ("m", 3)]
                sms, srs, rbcs = {}, {}, {}
                for op, ci in sig:
                    cs, sh, lcs = chunk(ci)
                    if op == "s":
                        sms[ci] = pool2.tile([1, csz], dt.float32, name="sm",
                                             tag="sm", bufs=2)
                        nc.vector.tensor_copy(sms[ci], pv[DK:DK + 1, cs])
                    elif op == "r":
                        srs[ci] = pool2.tile([1, csz], dt.float32, name="sr",
                                             tag="sr", bufs=2)
                        nc.vector.reciprocal_approx_fast(srs[ci], sms[ci])
                        rbcs[ci] = pool2.tile([DK, csz], dt.float32,
                                              name="rbc", tag="rbc", bufs=2)
                        nc.gpsimd.partition_broadcast(rbcs[ci], srs[ci])
                    else:
                        nc.vector.tensor_mul(
                            hot_sb[half_idx][sh][row0:row0 + DK, lcs],
                            pv[0:DK, cs], rbcs[ci])

        def attn_block(h):
            # attn_h @ [V | 1] interleaved per-t with scores of head h+1:
            # ACT stays saturated with exp work through the whole stream.
            # t OUTER on attnV so each exp tile is fully consumed after 4
            # back-to-back matmuls (releases the WAR for head h+1's exp
            # writes immediately -- no pipeline convoy). The 4 s-chunk
            # accumulators live in one 4-bank PSUM tile.
            pv = vpool.tile([DK + 1, S], dt.float32, name="av", tag="av")
            for t in range(16):
                for ch4 in range(4):
                    nc.tensor.matmul(
                        pv[:, ch4 * 512:(ch4 + 1) * 512],
                        lhsT=v_sb[t][:, h * VW:h * VW + DK + 1],
                        rhs=exp_sb[t][:, ch4 * 512:(ch4 + 1) * 512],
                        start=(t == 0), stop=(t == 15),
                    )
                if h + 1 < HPC:
                    scores_t(h + 1, t)
            norm_chunks(h, pv, 2 if h + 1 < HPC else 4)
            if h + 1 < HPC:
                ldw_fill(34)
            else:
                ldw_fill(12)
                dm = spool.tile([DK + 1, 512], dt.float32, name="mm",
                                tag="mm")
                dummies(dm, 20)

        def outproj():
            # ch outer so the first half of the out-projection starts as
            # soon as head 3's norm covers s < 1024
            for ch in range(2):
                for e in range(8):
                    ps = spool.tile([P, 1024], dt.float32, name="mm",
                                    tag="mm")
                    for half in range(2):
                        s0 = half * 512
                        for d2 in range(2):
                            nc.tensor.matmul(
                                ps[:, half * 512:(half + 1) * 512],
                                lhsT=wo_sb[d2][:, e * P:(e + 1) * P],
                                rhs=hot_sb[d2][ch][:, s0:s0 + 512],
                                start=(d2 == 0), stop=(d2 == 1),
                            )
                    st = pool2.tile([P, 1024], dt.bfloat16, name="st",
                                    tag="st", bufs=3)
                    if e % 2 == 0:
                        nc.vector.tensor_copy(st, ps)
                        nc.sync.dma_start(
                            out=outT_r[e][:, ch * 1024:(ch + 1) * 1024],
                            in_=st)
                    else:
                        nc.scalar.copy(st, ps)
                        nc.gpsimd.dma_start(
                            out=outT_r[e][:, ch * 1024:(ch + 1) * 1024],
                            in_=st)

        # Emission order: QKT d2=0 (Q before K so scores(0) unblocks after
        # three units), then a PE-dense stream [scores0 | V | QKT d2=1] that
        # puts exp work on ACT as early as possible, then the attention
        # blocks (scores of head h+1 ride inside head h's attnV).
        for qk in range(2):
            for ch in range(2):
                qkt_unit(0, ch, qk)
        for t in range(16):
            scores_t(0, t)
            vproj_t(t)
            if t % 4 == 0:
                u = t // 4
                qkt_unit(1, u // 2, u % 2)
        attn_block(0)
        attn_block(1)
        attn_block(2)
        attn_block(3)
        outproj()

    nc.compile()
    return nc


def _get_program():
    if "nc" not in _CACHE:
        _CACHE["nc"] = _build_program()
    return _CACHE["nc"]


def _shard_inputs(input, W_qkv, b_qkv, W_out):
    """Build the 8 per-core input maps (host-side shard + transpose + cast)."""
    in_maps = []
    xt_by_b = [
        np.ascontiguousarray(input[b].T).astype(BF16) for b in range(2)
    ]
    for core in range(N_CORES):
        b, g = divmod(core, HPC)
        cols = slice(g * DH, (g + 1) * DH)
        bq = (b_qkv[g * DH:(g + 1) * DH] / 8.0).astype(np.float32)
        bk = b_qkv[C + g * DH:C + (g + 1) * DH].astype(np.float32)
        bqk = np.stack([bq[:P], bq[P:], bk[:P], bk[P:]], axis=1)
        in_maps.append({
            "xt": xt_by_b[b],
            "wq": np.ascontiguousarray(W_qkv[:, cols] * 0.125).astype(BF16),
            "wk": np.ascontiguousarray(W_qkv[:, C:2 * C][:, cols]).astype(BF16),
            "wv": np.ascontiguousarray(W_qkv[:, 2 * C:][:, cols]).astype(BF16),
            "wo": np.ascontiguousarray(W_out[g * DH:(g + 1) * DH, :]).astype(BF16),
            "bqk": np.ascontiguousarray(bqk, dtype=np.float32),
            "bv": b_qkv[2 * C + g * DH:2 * C + (g + 1) * DH]
                  .astype(BF16).reshape(1, DH),
        })
    return in_maps


def kernel(input, W_qkv, b_qkv, W_out):
    from concourse.bass_utils import run_bass_kernel_spmd

    nc = _get_program()
    in_maps = _shard_inputs(
        np.asarray(input), np.asarray(W_qkv), np.asarray(b_qkv),
        np.asarray(W_out),
    )
    res = run_bass_kernel_spmd(nc, in_maps, core_ids=list(range(N_CORES)))
    out = np.zeros((2, S, C), dtype=np.float32)
    for core in range(N_CORES):
        b = core // HPC
        out[b] += np.asarray(res.results[core]["outT"]).astype(np.float32).T
    return out
